# revision 6
# baseline (speedup 1.0000x reference)
"""Differential attention kernel for Trainium2, 8 NeuronCores — bf16,
throttle-aware schedule.

Sharding: B(2) x head-groups(4) -> 8 cores; each core computes 3 heads'
differential attention for one batch element plus its partial slice of the
output projection (row-parallel over Wo). Host sums the 4 partials per batch
element and adds bo.

The HAM power manager clamps the PE to 4/8 duty after sustained dense
matmul phases (the old kernel lost ~100us to clamps after its projection
block and during its PE-only out-projection tail). This schedule dissolves
those phases into the ACT-bound attention windows:

  - lead-in: warmup + v-projection + qk-projection of head 0's two units
  - per (head, half) window: branch-interleaved score strips (bf16, one
    matmul per 512-col PSUM bank) -> exp on ACT -> PV. Branch 0's PV runs
    a strip behind its exp; branch 1's P tiles persist in SBUF and its PV
    runs at the window end, so the two branches share ONE PSUM accumulator
    (frees 2 banks vs the 2-accumulator layout).
  - the freed banks host an "aux" tile: interleaved qk-projection chunks
    for the next head and output-projection chunks for the previous
    window, emitted between score strips so the PE never runs a dense
    block and ACT never starves.
  - out-projection accumulates per-head partials into SBUF (bf16) via DVE,
    one [128,768] PSUM chunk at a time; tiles DMA out as head 2 lands.

PSUM: st0,st1 (2+2 banks) + u (2) + aux (2) = 8.
ACT (192 exps x [128,1024] ~= 214us) is the pacing engine.
"""

import os
import sys
from contextlib import ExitStack

for _p in ("/opt/trn_rl_repo", "/root/.axon_site/_ro/trn_rl_repo"):
    if os.path.isdir(_p) and _p not in sys.path:
        sys.path.insert(0, _p)

import ml_dtypes
import numpy as np

import concourse.bass as bass
import concourse.bacc as bacc_mod
import concourse.mybir as mybir
from concourse.bass_utils import run_bass_kernel_spmd
from concourse.tile import TileContext

BF16 = ml_dtypes.bfloat16
F = mybir.dt

B, N, C, H, D = 2, 2048, 768, 12, 64
HPC = 3          # heads per core
NCORES = 8
NT = N // 128    # 16 key strips / row tiles
QH = 1024        # q-window per (head, half)


def _body(nc, tc, ctx, xt, wqk, wv, wo, lamc, out):
    fp32, bf16 = F.float32, F.bfloat16
    Exp = mybir.ActivationFunctionType.Exp

    singles = ctx.enter_context(tc.tile_pool(name="singles", bufs=1))
    xt_sb = singles.tile([128, 6, N], bf16)       # x^T, c = ch*128+p
    wqk_sb = singles.tile([128, 6, 768], bf16)    # unit u at cols u*128..+128
    wv_sb = singles.tile([128, 6, HPC * D], bf16)
    wo_sb = singles.tile([64, HPC, C], bf16)      # per-head Wo rows
    lams_sb = singles.tile([128, 6], fp32)        # col u: 1.0 (br0) or -lam_h (br1)
    qkv_sb = singles.tile([128, 6, N], bf16)      # qT rows 0:64, kT rows 64:128
    k0_sb = singles.tile([64, 6, N], bf16)        # kT re-homed to partition base 0
    v_sb = singles.tile([128, NT, HPC, D + 1], bf16)
    pt1_sb = singles.tile([128, NT, QH], bf16)    # branch-1 P tiles (persist/window)
    diff_sb = singles.tile([64, HPC, N], bf16)
    oacc_sb = singles.tile([128, NT, C], bf16)    # out-proj partial accumulator
    # DRAM bounce for r: DMA-broadcast across partitions needs a DRAM source
    r_dram = nc.dram_tensor("r_bounce", [6, N], bf16)

    nc.sync.dma_start(out=wqk_sb, in_=wqk[:, :].rearrange("(ch p) w -> p ch w", p=128))
    nc.sync.dma_start(out=wv_sb, in_=wv[:, :].rearrange("(ch p) w -> p ch w", p=128))
    nc.sync.dma_start(out=wo_sb, in_=wo[:, :].rearrange("(h p) c -> p h c", p=64))
    nc.sync.dma_start(out=lams_sb, in_=lamc[:, :])
    xt_r = xt[:, :].rearrange("(ch p) n -> p ch n", p=128)
    for c in range(6):
        eng = nc.sync if c % 2 == 0 else nc.gpsimd
        eng.dma_start(out=xt_sb[:, c, :], in_=xt_r[:, c, :])
    nc.vector.memset(v_sb[:, :, :, D : D + 1], 1.0)

    # pre-warm the PE's HAM clock gate during the initial DMA wait so the
    # projection matmuls start at full clock (junk matmuls into scratch)
    with tc.tile_pool(name="warm_sb", bufs=1) as warm_sb, \
         tc.tile_pool(name="warm_ps", bufs=1, space="PSUM") as warm_ps:
        wsrc = warm_sb.tile([128, 512], bf16)
        nc.vector.memset(wsrc, 0.0)
        wt = warm_ps.tile([128, 512], fp32)
        for _ in range(24):
            nc.tensor.matmul(wt, lhsT=wsrc[:, 0:128], rhs=wsrc, start=True, stop=True)

    # ---------- lead-in projections: v (all heads) + qk units 0,1 ----------
    with tc.tile_pool(name="vpp", bufs=3, space="PSUM") as vpp:
        for ti in range(NT):
            vp = vpp.tile([128, HPC * D], fp32)
            for c in range(6):
                nc.tensor.matmul(
                    vp,
                    lhsT=xt_sb[:, c, ti * 128 : (ti + 1) * 128],
                    rhs=wv_sb[:, c, :],
                    start=(c == 0),
                    stop=(c == 5),
                )
            nc.vector.tensor_copy(
                v_sb[:, ti, :, 0:D], vp.rearrange("p (h d) -> p h d", h=HPC)
            )

    with tc.tile_pool(name="qpp", bufs=2, space="PSUM") as qpp:
        for u in range(2):
            pp = qpp.tile([128, N], fp32)
            for c in range(6):
                for g in range(4):
                    nc.tensor.matmul(
                        pp[:, g * 512 : (g + 1) * 512],
                        lhsT=wqk_sb[:, c, u * 128 : (u + 1) * 128],
                        rhs=xt_sb[:, c, g * 512 : (g + 1) * 512],
                        start=(c == 0),
                        stop=(c == 5),
                    )
            nc.vector.tensor_copy(qkv_sb[:, u, :], pp)
            nc.sync.dma_start(out=k0_sb[:, u, :], in_=qkv_sb[64:128, u, :])

    # ---------- attention windows with interleaved aux work ----------
    with tc.tile_pool(name="stp", bufs=1, space="PSUM") as stp, \
         tc.tile_pool(name="upp", bufs=1, space="PSUM") as upp, \
         tc.tile_pool(name="auxp", bufs=1, space="PSUM") as auxp, \
         tc.tile_pool(name="ptp", bufs=2) as ptp, \
         tc.tile_pool(name="rsc", bufs=1) as rsc, \
         tc.tile_pool(name="outp", bufs=2) as outp:

        def emit_qk_chunk(u, g):
            """One 512-col chunk of unit u's qk projection (aux PSUM)."""
            ck = auxp.tile([128, QH], fp32, tag="aux", name="ck")
            for c in range(6):
                nc.tensor.matmul(
                    ck[:, 0:512],
                    lhsT=wqk_sb[:, c, u * 128 : (u + 1) * 128],
                    rhs=xt_sb[:, c, g * 512 : (g + 1) * 512],
                    start=(c == 0),
                    stop=(c == 5),
                )
            nc.vector.tensor_copy(qkv_sb[:, u, g * 512 : (g + 1) * 512], ck[:, 0:512])
            nc.gpsimd.dma_start(
                out=k0_sb[:, u, g * 512 : (g + 1) * 512],
                in_=qkv_sb[64:128, u, g * 512 : (g + 1) * 512],
            )

        def emit_fo_chunk(hp, ti):
            """Head hp's out-proj contribution for row tile ti (aux PSUM),
            accumulated into oacc_sb; final head DMAs the tile out."""
            fo = auxp.tile([128, QH], fp32, tag="aux", name="fo")
            for o, w in ((0, 512), (512, 256)):
                nc.tensor.matmul(
                    fo[:, o : o + w],
                    lhsT=diff_sb[:, hp, ti * 128 : (ti + 1) * 128],
                    rhs=wo_sb[:, hp, o : o + w],
                    start=True,
                    stop=True,
                )
            if hp == 0:
                nc.vector.tensor_copy(oacc_sb[:, ti, :], fo[:, 0:C])
            else:
                nc.vector.tensor_add(oacc_sb[:, ti, :], oacc_sb[:, ti, :], fo[:, 0:C])
            if hp == HPC - 1:
                ot = outp.tile([128, C], bf16)
                nc.vector.tensor_copy(ot, oacc_sb[:, ti, :])
                oeng = nc.sync if ti % 2 == 0 else nc.gpsimd
                oeng.dma_start(out=out[ti * 128 : (ti + 1) * 128, :], in_=ot)

        def den_path(u_ps, u, q0, br):
            """Denominator (psum row 64) -> reciprocal -> r_dram; u rows ->
            bf16 stage tile. Returns the stage tile."""
            dsc = rsc.tile([65, QH], fp32, tag="dsc", name="dsc")
            nc.vector.tensor_copy(dsc[64:65, :], u_ps[64:65, :])
            den128 = rsc.tile([128, QH // 128], fp32, tag="dsc2", name="den128")
            nc.sync.dma_start(out=den128, in_=dsc[64:65, :])
            r128 = rsc.tile([128, QH // 128], fp32, tag="dsc3", name="r128")
            nc.vector.reciprocal(r128, den128)
            r128b = rsc.tile([128, QH // 128], bf16, tag="dsc4", name="r128b")
            nc.vector.tensor_scalar_mul(r128b, r128, lams_sb[:, u : u + 1])
            nc.sync.dma_start(out=r_dram[u : u + 1, q0 : q0 + QH], in_=r128b)
            us = rsc.tile([64, QH], bf16, tag=f"us{br}", name=f"us{br}")
            nc.vector.tensor_copy(us, u_ps[0:64, :])
            return us

        # aux schedule per window (h, half):
        #   qk chunks for unit 2*(h+1)+half at strips 1,3,5,7  (h < 2)
        #   fo chunks for the previous window at strips 8..15
        prev_win = None
        for h in range(HPC):
            for half in range(2):
                q0 = half * QH
                aux_items = []
                if h < HPC - 1:
                    un = 2 * (h + 1) + half
                    aux_items += [(1 + 2 * g, "qk", (un, g)) for g in range(4)]
                if prev_win is not None:
                    hp, halfp = prev_win
                    aux_items += [
                        (8 + tj, "fo", (hp, halfp * (NT // 2) + tj))
                        for tj in range(NT // 2)
                    ]
                aux_by_slot = {s: (kind, args) for s, kind, args in aux_items}

                u_ps = upp.tile([65, QH], fp32, tag="u", name="u_ps0")
                pt_prev = None
                for ti in range(NT):
                    pt0 = None
                    for br in range(2):
                        u = 2 * h + br
                        st = stp.tile([128, QH], fp32, tag=f"st{br}", name=f"st{br}")
                        for g in range(2):
                            nc.tensor.matmul(
                                st[:, g * 512 : (g + 1) * 512],
                                lhsT=k0_sb[:, u, ti * 128 : (ti + 1) * 128],
                                rhs=qkv_sb[0:64, u, q0 + g * 512 : q0 + (g + 1) * 512],
                                start=True,
                                stop=True,
                            )
                        if br == 0:
                            pt0 = ptp.tile([128, QH], bf16, tag="pt0", name="pt0")
                            nc.scalar.activation(pt0, st, Exp)
                        else:
                            nc.scalar.activation(pt1_sb[:, ti, :], st, Exp)
                    # branch-0 PV one strip behind its exp
                    if ti > 0:
                        for g in range(2):
                            nc.tensor.matmul(
                                u_ps[:, g * 512 : (g + 1) * 512],
                                lhsT=v_sb[:, ti - 1, h, :],
                                rhs=pt_prev[:, g * 512 : (g + 1) * 512],
                                start=(ti - 1 == 0),
                                stop=False,
                            )
                    pt_prev = pt0
                    if ti in aux_by_slot:
                        kind, args = aux_by_slot[ti]
                        if kind == "qk":
                            emit_qk_chunk(*args)
                        else:
                            emit_fo_chunk(*args)
                for g in range(2):
                    nc.tensor.matmul(
                        u_ps[:, g * 512 : (g + 1) * 512],
                        lhsT=v_sb[:, NT - 1, h, :],
                        rhs=pt_prev[:, g * 512 : (g + 1) * 512],
                        start=False,
                        stop=True,
                    )
                us0 = den_path(u_ps, 2 * h, q0, 0)

                # branch-1 PV from the persisted P tiles, reusing the u banks
                u_ps1 = upp.tile([65, QH], fp32, tag="u", name="u_ps1")
                for ti in range(NT):
                    for g in range(2):
                        nc.tensor.matmul(
                            u_ps1[:, g * 512 : (g + 1) * 512],
                            lhsT=v_sb[:, ti, h, :],
                            rhs=pt1_sb[:, ti, g * 512 : (g + 1) * 512],
                            start=(ti == 0),
                            stop=(ti == NT - 1),
                        )
                us1 = den_path(u_ps1, 2 * h + 1, q0, 1)

                # diff = u0*R0 + u1*R1 (-lam_h folded into branch-1's r)
                rb0 = rsc.tile([64, QH], bf16, tag="rb0", name="rb0")
                nc.sync.dma_start(
                    out=rb0,
                    in_=r_dram[2 * h : 2 * h + 1, q0 : q0 + QH].partition_broadcast(64),
                )
                rb1 = rsc.tile([64, QH], bf16, tag="rb1", name="rb1")
                nc.sync.dma_start(
                    out=rb1,
                    in_=r_dram[2 * h + 1 : 2 * h + 2, q0 : q0 + QH].partition_broadcast(64),
                )
                t1 = rsc.tile([64, QH], bf16, tag="t1", name="t1")
                nc.vector.tensor_mul(t1, us0, rb0)
                t2 = rsc.tile([64, QH], bf16, tag="t2", name="t2")
                nc.vector.tensor_mul(t2, us1, rb1)
                nc.vector.tensor_add(diff_sb[:, h, q0 : q0 + QH], t1, t2)
                prev_win = (h, half)

        # tail: out-proj for the last window
        hp, halfp = prev_win
        for tj in range(NT // 2):
            emit_fo_chunk(hp, halfp * (NT // 2) + tj)


def build_bass():
    nc = bacc_mod.Bacc(None)
    xt = nc.dram_tensor("xt", [C, N], F.bfloat16, kind="ExternalInput")
    wqk = nc.dram_tensor("wqk", [C, 768], F.bfloat16, kind="ExternalInput")
    wv = nc.dram_tensor("wv", [C, HPC * D], F.bfloat16, kind="ExternalInput")
    wo = nc.dram_tensor("wo", [HPC * D, C], F.bfloat16, kind="ExternalInput")
    lamc = nc.dram_tensor("lamc", [128, 6], F.float32, kind="ExternalInput")
    out = nc.dram_tensor("out", [N, C], F.bfloat16, kind="ExternalOutput")
    with TileContext(nc) as tc:
        with ExitStack() as ctx:
            _body(nc, tc, ctx, xt, wqk, wv, wo, lamc, out)
    nc.compile()
    return nc


_NC = None


def _get_nc():
    global _NC
    if _NC is None:
        _NC = build_bass()
    return _NC


def _prep_core(core, x, Wq, Wk, Wv, Wo, lam):
    b = core // 4
    heads = [(core % 4) * HPC + i for i in range(HPC)]
    sc = 1.0 / np.sqrt(D)
    xt = np.ascontiguousarray(x[b].T).astype(BF16)
    wqk = np.empty((C, 768), np.float32)
    for i, h in enumerate(heads):
        for br in range(2):
            u = 2 * i + br
            wqk[:, u * 128 : u * 128 + 64] = Wq[:, br * C + h * D : br * C + (h + 1) * D] * sc
            wqk[:, u * 128 + 64 : (u + 1) * 128] = Wk[:, br * C + h * D : br * C + (h + 1) * D]
    wv = np.concatenate([Wv[:, h * D : (h + 1) * D] for h in heads], axis=1)
    wo = np.concatenate([Wo[h * D : (h + 1) * D, :] for h in heads], axis=0)
    lams = np.zeros((128, 6), np.float32)
    for i, h in enumerate(heads):
        lams[:, 2 * i] = 1.0
        lams[:, 2 * i + 1] = -lam[h]
    return dict(
        xt=xt,
        wqk=wqk.astype(BF16),
        wv=wv.astype(BF16),
        wo=wo.astype(BF16),
        lamc=lams,
    )


def kernel(x, Wq, Wk, Wv, lambda_p, Wo, bo, _trace=False, _tmpdir=None):
    x = np.asarray(x, np.float32)
    lam = np.exp(np.asarray(lambda_p, np.float32).reshape(H))
    in_maps = [
        _prep_core(core, x, np.asarray(Wq, np.float32), np.asarray(Wk, np.float32),
                   np.asarray(Wv, np.float32), np.asarray(Wo, np.float32), lam)
        for core in range(NCORES)
    ]
    nc = _get_nc()
    res = run_bass_kernel_spmd(
        nc, in_maps, list(range(NCORES)), trace=_trace, tmpdir=_tmpdir
    )
    outf = np.zeros((B, N, C), np.float32)
    for core in range(NCORES):
        outf[core // 4] += np.asarray(res.results[core]["out"], np.float32)
    outf += np.asarray(bo, np.float32)[None, None, :]
    if _trace:
        kernel.last_exec_time_ns = res.exec_time_ns
    return outf
